# revision 1
# baseline (speedup 1.0000x reference)
"""RoIAlign (scale_and_translate, linear, antialias) Trainium2 kernel.

Strategy: channel-shard across 8 NeuronCores. Each core receives a
contiguous [512, 512, 8] slice of the feature map (kept resident in
SBUF), plus host-precomputed dense resampling weight matrices
Wy/Wx [512, 32] per box, and computes all 512 boxes for its 8 channels:

  stage 1 (PE):  T[i, x, c]   = sum_y Wy[y, i] * F[y, x, c]   (per 128-px x-tile)
  transpose:     SBUF->SBUF DMA rearrange  T -> rhs2[x, (i, c)]
  stage 2 (PE):  out[j, i, c] = sum_x Wx[x, j] * T[i, x, c]

Device output layout is [n, j, i, c]; the host transposes to
[n, i, j, c] and concatenates channel shards.
"""

import numpy as np

H = 512
W = 512
C = 64
N_BOXES = 512
OUT = 32
N_CORES = 8
C_LOC = C // N_CORES  # 8 channels per core
PART = 128


# ---------------------------------------------------------------------------
# Host-side weight computation (mirrors jax.image.scale_and_translate with
# method="linear", antialias=True)
# ---------------------------------------------------------------------------

def _compute_weight_mat(in_size, out_size, scale, translation):
    inv_scale = 1.0 / scale
    kernel_scale = max(inv_scale, 1.0)
    sample_f = (np.arange(out_size, dtype=np.float64) + 0.5) * inv_scale \
        - translation * inv_scale - 0.5
    x = np.abs(sample_f[None, :] - np.arange(in_size, dtype=np.float64)[:, None]) \
        / kernel_scale
    weights = np.maximum(0.0, 1.0 - x)
    total = weights.sum(axis=0, keepdims=True)
    weights = np.where(
        np.abs(total) > 1000.0 * float(np.finfo(np.float32).eps),
        weights / np.where(total != 0, total, 1.0),
        0.0,
    )
    valid = (sample_f >= -0.5) & (sample_f <= in_size - 0.5)
    return np.where(valid[None, :], weights, 0.0).astype(np.float32)


def host_geometry(boxes):
    """Per-box dense weights + extents.

    Returns wy_all [N, 512, OUT], wx_all [N, 512, OUT] fp32 and a list of
    per-box geometry dicts.
    """
    boxes = np.asarray(boxes, dtype=np.float64)
    wy_all = np.zeros((N_BOXES, H, OUT), np.float32)
    wx_all = np.zeros((N_BOXES, W, OUT), np.float32)
    geoms = []
    for n in range(N_BOXES):
        cx, cy, w, h = boxes[n]
        x0 = cx - w / 2
        y0 = cy - h / 2
        w = max(w, 1e-6)
        h = max(h, 1e-6)
        x_scale = OUT / (w * W)
        y_scale = OUT / (h * H)
        ty = -y0 * OUT / h
        tx = -x0 * OUT / w
        wy = _compute_weight_mat(H, OUT, y_scale, ty)
        wx = _compute_weight_mat(W, OUT, x_scale, tx)
        wy_all[n] = wy
        wx_all[n] = wx

        ynz = np.nonzero(wy.any(axis=1))[0]
        xnz = np.nonzero(wx.any(axis=1))[0]
        if len(ynz) == 0 or len(xnz) == 0:
            geoms.append(None)
            continue
        r0, r1 = int(ynz[0]), int(ynz[-1]) + 1
        c0, c1 = int(xnz[0]), int(xnz[-1]) + 1
        # 128-row/col tile footprints. matmul K always spans partitions
        # [0, 128) of a tile: zero weights outside the band contribute
        # nothing and K depth is free on PE.
        ts = list(range(r0 // PART, (r1 - 1) // PART + 1))
        xts = list(range(c0 // PART, (c1 - 1) // PART + 1))
        geoms.append({"ts": ts, "xts": xts})
    return wy_all, wx_all, geoms


def sort_boxes(geoms):
    """Group boxes with similar tile footprints so grouped matmuls share
    stationary F loads. Returns a permutation (sorted -> original index)."""
    def key(n):
        g = geoms[n]
        if g is None:
            return (9, 9, 9, 9)
        return (g["ts"][0], g["ts"][-1], g["xts"][0], g["xts"][-1])
    return sorted(range(N_BOXES), key=key)


def reference_numpy(feature_map, wy_all, wx_all):
    """Two-stage numpy computation used to validate the host weights."""
    out = np.zeros((N_BOXES, OUT, OUT, feature_map.shape[2]), np.float32)
    f = feature_map
    for n in range(N_BOXES):
        t = np.einsum("yi,yxc->ixc", wy_all[n], f)
        out[n] = np.einsum("xj,ixc->ijc", wx_all[n], t)
    return out


# ---------------------------------------------------------------------------
# Device program
# ---------------------------------------------------------------------------

def _split_multiwait_bir(raw: bytes) -> bytes:
    """The walrus build here accepts only one sync wait per instruction.
    Hoist extra waits onto single-wait EventSemaphore instructions inserted
    just before, on the same engine (per-engine order is preserved)."""
    import orjson

    d = orjson.loads(raw)
    ctr = 0
    for fn in d.get("functions", []):
        for bb in fn.get("blocks") or []:
            out = []
            for ins in bb["instructions"]:
                si = ins.get("sync_info")
                ws = (si or {}).get("on_wait") or []
                if len(ws) > 1:
                    for w in ws[:-1]:
                        ctr += 1
                        out.append({
                            "debug": ins.get("debug", 0),
                            "engine": ins["engine"],
                            "ins": [],
                            "outs": [],
                            "name": f"{ins['name']}-xw{ctr}",
                            "opcode": "EventSemaphore",
                            "sync_info": {"on_update": [], "on_wait": [w]},
                        })
                    si["on_wait"] = [ws[-1]]
                out.append(ins)
            bb["instructions"] = out
    return orjson.dumps(d)


def _patch_serialization(nc):
    orig = nc.to_json_bytes

    def patched():
        return _split_multiwait_bir(orig())

    nc.to_json_bytes = patched
    return nc

def _build_program(geoms, perm, repeat=1, timing=False, ablate=()):
    """geoms: per original-box geometry; perm: sorted->original index map.
    Device processes boxes in sorted order, in groups of GRP; outputs go to
    the sorted slot k (host unpermutes). repeat>1 re-runs the whole box
    loop (identical outputs) for wall-clock slope timing. timing=True
    shrinks the DRAM weight/output tensors (device work unchanged, host
    upload/download tiny) -- results are garbage, only for timing."""
    import concourse.bass as bass
    import concourse.mybir as mybir
    import concourse.tile as tile
    from concourse.vector_clock import ScopedClock
    import bass_rust

    class TC(tile.TileContext):
        """TileContext with the tail drain's multi-sem wait split into
        individual single-wait instructions (this walrus rejects >1 wait
        on a CTRL instruction)."""

        def _drain_and_barrier(self, tick_clock, wait_clock):
            nc = self.nc
            probe = nc.sync.drain()
            wait_clock.add_sem_waits(
                probe.ins, ScopedClock({None: tick_clock.global_clock})
            )
            waits = list(probe.ins.sync_info.on_wait)
            probe.ins.sync_info = bass_rust.SyncInfo(on_wait=[], on_update=[])
            by_name = {hh.name: hh for hh in self.sems.allocated().values()}
            for wt in waits:
                nc.sync.wait_ge(by_name[wt.ant_name], wt.wait_value)
            nc.all_engine_barrier()
            popped = nc._tile_sem_poison_stack.pop()
            assert popped is self._sem_poison
            nc.clear_and_free_semaphores(list(self.sems.allocated().values()))
            nc.all_engine_barrier()

    FP32 = mybir.dt.float32
    BF16 = mybir.dt.bfloat16
    nc = bass.Bass()
    # f is channel-major [c, y, x] bf16 so per-(c, y-tile) lhsT slices have
    # contiguous x columns (FWL-friendly weight loads).
    f_d = nc.dram_tensor("f", [C_LOC, H, W], BF16, kind="ExternalInput")
    # wy/wx are pre-sorted by the host (sorted box order)
    NW = 16 if timing else N_BOXES
    NO = 4 if timing else N_BOXES
    wy_d = nc.dram_tensor("wy", [NW, H, OUT], BF16, kind="ExternalInput")
    wx_d = nc.dram_tensor("wx", [NW, W, OUT], FP32, kind="ExternalInput")
    # device output layout [sorted_n, j, c, i]
    out_d = nc.dram_tensor("out", [NO, OUT, C_LOC, OUT], FP32,
                           kind="ExternalOutput")

    NT = H // PART          # 4 y/x partition tiles
    GRP = 4                 # boxes per group (share stationary-F loads)
    GCI = GRP * OUT         # 128 free cols per c-region of group psum_t
    CI = C_LOC * OUT        # 256 free cols per box in psum2
    CHUNK = 16              # boxes per weight-DMA chunk
    FREE_T = C_LOC * GRP * OUT   # 1024: psum_t free size (c, b, i)
    FREE_O = GRP * C_LOC * OUT   # 1024: psum2 free size (b, c, i)

    # group geometry (sorted order)
    groups = []
    for g0 in range(0, N_BOXES, GRP):
        members = []
        for k in range(g0, min(g0 + GRP, N_BOXES)):
            g = geoms[perm[k]]
            if g is not None:
                members.append((k - g0, g))
        if not members:
            continue
        union_t = sorted({t for _, g in members for t in g["ts"]})
        union_x = sorted({x for _, g in members for x in g["xts"]})
        groups.append((g0, members, union_t, union_x))

    from contextlib import ExitStack

    with TC(nc) as tc, ExitStack() as ctx:
        fpool = ctx.enter_context(tc.tile_pool(name="fmap", bufs=1))
        wpool = ctx.enter_context(tc.tile_pool(name="wts", bufs=2))
        rpool = ctx.enter_context(tc.tile_pool(name="rhs2", bufs=6))
        opool = ctx.enter_context(tc.tile_pool(name="osb", bufs=4))
        p1pool = ctx.enter_context(tc.tile_pool(name="psumT", bufs=2, space="PSUM"))
        p2pool = ctx.enter_context(tc.tile_pool(name="psum2", bufs=2, space="PSUM"))

        # resident feature map slice: [128, (c, t, x)], y = t*128 + p
        f_sb = fpool.tile([PART, C_LOC * NT * W], BF16)
        f_v = f_sb[:].rearrange("p (c t x) -> p c t x", c=C_LOC, t=NT)
        nc.sync.dma_start(
            out=f_v,
            in_=f_d.rearrange("c (t p) x -> p c t x", p=PART),
        )

        evac_flip = 0
        cur_chunk = -1
        wy_v = wx_v = None
        for (g0, members, union_t, union_x) in groups * repeat:
            chunk = g0 // CHUNK
            if chunk != cur_chunk:
                cur_chunk = chunk
                b0 = 0 if timing else chunk * CHUNK
                wy_sb = wpool.tile([PART, CHUNK * NT * OUT], BF16, tag="wy")
                wy_v = wy_sb[:].rearrange("p (b t i) -> p b t i", b=CHUNK, t=NT)
                wx_sb = wpool.tile([PART, CHUNK * NT * OUT], FP32, tag="wx")
                wx_v = wx_sb[:].rearrange("p (b t i) -> p b t i", b=CHUNK, t=NT)
                if "wdma" not in ablate:
                    nc.sync.dma_start(
                        out=wy_v,
                        in_=wy_d[b0:b0 + CHUNK].rearrange(
                            "b (t p) i -> p b t i", p=PART),
                    )
                    nc.sync.dma_start(
                        out=wx_v,
                        in_=wx_d[b0:b0 + CHUNK].rearrange(
                            "b (t p) i -> p b t i", p=PART),
                    )
            bl0 = g0 - chunk * CHUNK  # group's first box within the chunk

            psum2 = p2pool.tile([OUT, FREE_O], FP32)
            # stage 1 for all x-tiles of the group first; T^T tiles stay
            # in SBUF so each box's stage-2 PSUM group runs uninterleaved
            # (only one accumulation group may be open per PSUM bank).
            r_views = []
            for xt in union_x:
                # T^T[x, (c, b, i)] += F[y,x,c] * Wy[y,(b,i)]
                psum_t = p1pool.tile([PART, FREE_T], FP32)
                xsl = slice(xt * PART, (xt + 1) * PART)
                if "stage1" not in ablate:
                    for c in range(C_LOC):
                        for si, t in enumerate(union_t):
                            nc.tensor.matmul(
                                out=psum_t[:, c * GCI:(c + 1) * GCI],
                                lhsT=f_v[:, c, t, xsl],
                                rhs=wy_v[:, bl0:bl0 + GRP, t, :],
                                start=(si == 0),
                                stop=(si == len(union_t) - 1),
                            )
                # evacuate PSUM -> SBUF (alternate DVE / ACT)
                if "evac" not in ablate:
                    rhs2 = rpool.tile([PART, FREE_T], FP32)
                    if evac_flip & 1:
                        nc.scalar.copy(rhs2[:], psum_t[:])
                    else:
                        nc.vector.tensor_copy(out=rhs2[:], in_=psum_t[:])
                    evac_flip += 1
                    r_views.append(
                        rhs2[:].rearrange("x (c b i) -> x c b i", c=C_LOC, b=GRP))
            # stage 2: every member accumulates over the union x-set; Wx is
            # zero outside a box's own extent so extra tiles contribute 0.
            if "stage2" not in ablate:
                for bb, g in members:
                    for kx, xt in enumerate(union_x):
                        nc.tensor.matmul(
                            out=psum2[:, bb * CI:(bb + 1) * CI],
                            lhsT=wx_v[:, bl0 + bb, xt, :],
                            rhs=r_views[kx][:, :, bb, :],
                            start=(kx == 0),
                            stop=(kx == len(union_x) - 1),
                        )
            if "oevac" not in ablate:
                o_sb = opool.tile([OUT, FREE_O], FP32)
                if evac_flip & 1:
                    nc.scalar.copy(o_sb[:], psum2[:])
                else:
                    nc.vector.tensor_copy(out=o_sb[:], in_=psum2[:])
                evac_flip += 1
                if "odma" not in ablate:
                    o0 = 0 if timing else g0
                    nc.sync.dma_start(
                        out=out_d[o0:o0 + GRP].rearrange("b j c i -> j b (c i)"),
                        in_=o_sb[:].rearrange("j (b ci) -> j b ci", b=GRP),
                    )
    return _patch_serialization(nc)


# ---------------------------------------------------------------------------
# Entry point
# ---------------------------------------------------------------------------

_LAST = {}


def kernel(feature_map, boxes, output_width):
    from concourse.bass_utils import run_bass_kernel_spmd

    feature_map = np.asarray(feature_map, dtype=np.float32)
    boxes_np = np.asarray(boxes, dtype=np.float32)
    assert int(output_width) == OUT

    wy_all, wx_all, geoms = host_geometry(boxes_np)
    perm = sort_boxes(geoms)
    nc = _build_program(geoms, perm)

    import ml_dtypes
    wy_bf = np.ascontiguousarray(wy_all[perm]).astype(ml_dtypes.bfloat16)
    wx_srt = np.ascontiguousarray(wx_all[perm])
    in_maps = []
    for k in range(N_CORES):
        # channel-major [c, y, x] bf16 slice
        f_k = np.ascontiguousarray(
            feature_map[:, :, k * C_LOC:(k + 1) * C_LOC].transpose(2, 0, 1)
        ).astype(ml_dtypes.bfloat16)
        in_maps.append({"f": f_k, "wy": wy_bf, "wx": wx_srt})

    _LAST["nc"] = nc
    _LAST["in_maps"] = in_maps
    res = run_bass_kernel_spmd(nc, in_maps, list(range(N_CORES)))

    out = np.empty((N_BOXES, OUT, OUT, C), np.float32)
    perm_np = np.asarray(perm)
    for k in range(N_CORES):
        # device layout [sorted_n, j, c, i] -> original order [n, i, j, c]
        dev = res.results[k]["out"].transpose(0, 3, 1, 2)
        out[perm_np, :, :, k * C_LOC:(k + 1) * C_LOC] = dev
    return out


def estimate_hw_ns():
    """Cost-model estimate of the per-core kernel duration (ns)."""
    from concourse.timeline_sim import TimelineSim
    nc = _LAST.get("nc")
    if nc is None:
        return -1
    sim = TimelineSim(nc)
    sim.simulate()
    return int(sim.time)


def measure_wall(n=5):
    """Wall-clock of repeated dispatches (includes axon round trips)."""
    import time
    from concourse.bass_utils import run_bass_kernel_spmd
    times = []
    for _ in range(n):
        t0 = time.perf_counter()
        run_bass_kernel_spmd(_LAST["nc"], _LAST["in_maps"], list(range(N_CORES)))
        times.append(time.perf_counter() - t0)
    return times

